# revision 9
# baseline (speedup 1.0000x reference)
import os
import numpy as np

# nn_PixelflyLinear: y = (x @ w1.T) @ w2.T + b + butterfly_matmul(x, weight, flat_idx)
# Data-parallel over tokens: 8 cores x 512 tokens, weights replicated.
# Device computes yT (out_f on partitions, tokens on free dim); host transposes.

TOKENS, IN_F, OUT_F, LOWRANK = 4096, 4096, 4096, 256
BLOCK, ACTIVE, NB = 256, 5, 16
NCORES = 8
TPC = TOKENS // NCORES          # 512 tokens per core
NG = OUT_F // 128               # 32 output half-block groups
NXT = IN_F // 128               # 32 input tiles
NSLOT = 12                      # 10 butterfly + 2 lowrank lhsT slots per group

_CACHE = {}
LAST = {"exec_time_ns": None}


def _derive_xtile_idx(flat):
    xtile_idx = np.zeros((NG, 10), np.int64)
    for ob in range(NB):
        for j in range(ACTIVE):
            m = int(flat[ob, j])
            q = m // ACTIVE
            for rh in range(2):
                for kh in range(2):
                    xtile_idx[ob * 2 + rh, j * 2 + kh] = q * 2 + kh
    return xtile_idx


def _build(xtile_idx):
    import concourse.bacc as bacc
    import concourse.mybir as mybir
    import concourse.tile as tile

    nc = bacc.Bacc("TRN2", target_bir_lowering=False, debug=False,
                   num_devices=NCORES)
    dt = mybir.dt

    xpack_d = nc.dram_tensor("xpack", [NXT, 128, TPC], dt.float16,
                             kind="ExternalInput")
    w1_d = nc.dram_tensor("w1pack", [NXT, 128, 256], dt.float16,
                          kind="ExternalInput")
    g_d = nc.dram_tensor("gpack", [NG, 128, NSLOT * 128], dt.float16,
                         kind="ExternalInput")
    b_d = nc.dram_tensor("bpack", [128, NG], dt.float32, kind="ExternalInput")
    y_d = nc.dram_tensor("y", [NG, 128, TPC], dt.float32,
                         kind="ExternalOutput")

    with tile.TileContext(nc) as tc:
        with (
            tc.tile_pool(name="res", bufs=1) as res_pool,
            tc.tile_pool(name="gstream", bufs=3) as gpool,
            tc.tile_pool(name="ypool", bufs=4) as ypool,
            tc.tile_pool(name="upsum", bufs=1, space="PSUM") as upsum,
            tc.tile_pool(name="gpsum", bufs=6, space="PSUM") as gpsum,
        ):
            bt = res_pool.tile([128, NG], dt.float32, tag="b")
            nc.sync.dma_start(bt[:], b_d[:])
            xts, w1ts = [], []
            for i in range(NXT):
                xt = res_pool.tile([128, TPC], dt.float16, tag=f"x{i}",
                                   name=f"xt{i}")
                nc.sync.dma_start(xt[:], xpack_d[i])
                xts.append(xt)
                w1t = res_pool.tile([128, 256], dt.float16, tag=f"w1_{i}",
                                    name=f"w1t{i}")
                nc.sync.dma_start(w1t[:], w1_d[i])
                w1ts.append(w1t)

            # u = w1 @ xT, accumulated over the 32 input tiles as they land
            u_ps = [upsum.tile([128, TPC], dt.float32, tag=f"u{lh}",
                               name=f"ups{lh}") for lh in range(2)]
            for i in range(NXT):
                for lh in range(2):
                    nc.tensor.matmul(u_ps[lh][:],
                                     w1ts[i][:, lh * 128:(lh + 1) * 128],
                                     xts[i][:],
                                     start=(i == 0), stop=(i == NXT - 1))
            u_sb = []
            for lh in range(2):
                ut = res_pool.tile([128, TPC], dt.float16, tag=f"usb{lh}",
                                   name=f"usb{lh}")
                nc.vector.tensor_copy(ut[:], u_ps[lh][:])
                u_sb.append(ut)

            for g in range(NG):
                gt = gpool.tile([128, NSLOT * 128], dt.float16, tag="g",
                                name=f"gt{g}")
                nc.sync.dma_start(gt[:], g_d[g])
                acc = gpsum.tile([128, TPC], dt.float32, tag="acc",
                                 name=f"acc{g}")
                for s in range(10):
                    nc.tensor.matmul(acc[:], gt[:, s * 128:(s + 1) * 128],
                                     xts[int(xtile_idx[g, s])][:],
                                     start=(s == 0), stop=False)
                for lh in range(2):
                    nc.tensor.matmul(acc[:],
                                     gt[:, (10 + lh) * 128:(11 + lh) * 128],
                                     u_sb[lh][:],
                                     start=False, stop=(lh == 1))
                yt = ypool.tile([128, TPC], dt.float32, tag="y",
                                name=f"yt{g}")
                nc.vector.tensor_scalar_add(yt[:], acc[:], bt[:, g:g + 1])
                nc.sync.dma_start(y_d[g], yt[:])

    nc.compile()
    return nc


def _pack_weights(weight, w1, w2, b, flat):
    r2 = np.arange(BLOCK)
    gpack = np.empty((NG, 128, NSLOT * 128), np.float16)
    for ob in range(NB):
        for j in range(ACTIVE):
            m = int(flat[ob, j])
            q, a2 = m // ACTIVE, m % ACTIVE
            k = a2 * BLOCK + r2
            Wblk = weight[q * BLOCK + k // ACTIVE, k % ACTIVE, :]  # [r2, c]
            for rh in range(2):
                g = ob * 2 + rh
                for kh in range(2):
                    s = j * 2 + kh
                    gpack[g, :, s * 128:(s + 1) * 128] = \
                        Wblk[rh * 128:(rh + 1) * 128,
                             kh * 128:(kh + 1) * 128].T
    for g in range(NG):
        for lh in range(2):
            s = 10 + lh
            gpack[g, :, s * 128:(s + 1) * 128] = \
                w2[g * 128:(g + 1) * 128, lh * 128:(lh + 1) * 128].T
    w1sb = np.ascontiguousarray(
        w1.reshape(2, 128, 32, 128).transpose(2, 3, 0, 1)
          .reshape(NXT, 128, 256)).astype(np.float16)
    bpack = np.ascontiguousarray(b.reshape(NG, 128).T)
    return gpack, w1sb, bpack


def _ensure_axon_hooks():
    # Some images lack antenv.axon_hooks; bass_utils imports it on the
    # trace path. Provide a stub so trace degrades gracefully.
    import sys
    import types
    try:
        import antenv.axon_hooks  # noqa: F401
        return
    except ImportError:
        pass
    mod = types.ModuleType("antenv.axon_hooks")
    mod._hook = None
    mod.set_axon_ntff_profile_hook = lambda h: setattr(mod, "_hook", h)
    mod.get_axon_ntff_profile_hook = lambda: mod._hook
    sys.modules["antenv.axon_hooks"] = mod
    try:
        import antenv
        antenv.axon_hooks = mod
    except ImportError:
        pass


def kernel(x, weight, w1, w2, b, butterfly_flat_indices):
    _ensure_axon_hooks()
    from concourse.bass_utils import run_bass_kernel_spmd

    x = np.ascontiguousarray(x, np.float32)
    weight = np.ascontiguousarray(weight, np.float32)
    w1 = np.ascontiguousarray(w1, np.float32)
    w2 = np.ascontiguousarray(w2, np.float32)
    b = np.ascontiguousarray(b, np.float32)
    flat = np.asarray(butterfly_flat_indices)

    xtile_idx = _derive_xtile_idx(flat)
    key = xtile_idx.tobytes()
    if key not in _CACHE:
        _CACHE[key] = _build(xtile_idx)
    nc = _CACHE[key]

    gpack, w1sb, bpack = _pack_weights(weight, w1, w2, b, flat)
    in_maps = []
    for c in range(NCORES):
        xs = x[c * TPC:(c + 1) * TPC]
        xpack = np.ascontiguousarray(
            xs.T.reshape(NXT, 128, TPC)).astype(np.float16)
        in_maps.append({"xpack": xpack, "w1pack": w1sb, "gpack": gpack,
                        "bpack": bpack})

    trace = bool(int(os.environ.get("PIXELFLY_TRACE", "0")))
    res = run_bass_kernel_spmd(nc, in_maps, list(range(NCORES)), trace=trace)
    LAST["exec_time_ns"] = res.exec_time_ns
    LAST["results"] = res

    out = np.empty((TOKENS, OUT_F), np.float32)
    for c in range(NCORES):
        yc = res.results[c]["y"]  # [NG, 128, TPC]
        out[c * TPC:(c + 1) * TPC] = yc.reshape(OUT_F, TPC).T
    return out


# revision 15
# speedup vs baseline: 1.1772x; 1.1772x over previous
import os
import numpy as np

# nn_PixelflyLinear: y = (x @ w1.T) @ w2.T + b + butterfly_matmul(x, weight, flat_idx)
# Data-parallel over tokens: 8 cores x 512 tokens, weights replicated.
# Device computes yT (out_f on partitions, tokens on free dim); host transposes.

TOKENS, IN_F, OUT_F, LOWRANK = 4096, 4096, 4096, 256
BLOCK, ACTIVE, NB = 256, 5, 16
NCORES = 8
TPC = TOKENS // NCORES          # 512 tokens per core
NG = OUT_F // 128               # 32 output half-block groups
NXT = IN_F // 128               # 32 input tiles
NSLOT = 12                      # 10 butterfly + 2 lowrank lhsT slots per group

_CACHE = {}
LAST = {"exec_time_ns": None}


def _derive_xtile_idx(flat):
    xtile_idx = np.zeros((NG, 10), np.int64)
    for ob in range(NB):
        for j in range(ACTIVE):
            m = int(flat[ob, j])
            q = m // ACTIVE
            for rh in range(2):
                for kh in range(2):
                    xtile_idx[ob * 2 + rh, j * 2 + kh] = q * 2 + kh
    return xtile_idx


def _build(xtile_idx):
    import concourse.bacc as bacc
    import concourse.mybir as mybir
    import concourse.tile as tile

    nc = bacc.Bacc("TRN2", target_bir_lowering=False, debug=False,
                   num_devices=NCORES)
    dt = mybir.dt

    xpack_d = nc.dram_tensor("xpack", [NXT, 128, TPC], dt.float16,
                             kind="ExternalInput")
    w1_d = nc.dram_tensor("w1pack", [128, 64 * 128], dt.float16,
                          kind="ExternalInput")
    g_d = nc.dram_tensor("gpack", [NG, 128, NSLOT * 128], dt.float16,
                         kind="ExternalInput")
    b_d = nc.dram_tensor("bpack", [128, NG], dt.float32, kind="ExternalInput")
    y_d = nc.dram_tensor("y", [NG, 128, TPC], dt.float16,
                         kind="ExternalOutput")

    with tile.TileContext(nc) as tc:
        with (
            tc.tile_pool(name="res", bufs=1) as res_pool,
            tc.tile_pool(name="gstream", bufs=6) as gpool,
            tc.tile_pool(name="ypool", bufs=6) as ypool,
            tc.tile_pool(name="upsum", bufs=1, space="PSUM") as upsum,
            tc.tile_pool(name="gpsum", bufs=6, space="PSUM") as gpsum,
        ):
            bt = res_pool.tile([128, NG], dt.float32, tag="b")
            nc.sync.dma_start(bt[:], b_d[:])
            w1t = res_pool.tile([128, 64 * 128], dt.float16, tag="w1",
                                name="w1t")
            nc.sync.dma_start(w1t[:], w1_d[:])
            xts = []
            for i in range(NXT):
                xt = res_pool.tile([128, TPC], dt.float16, tag=f"x{i}",
                                   name=f"xt{i}")
                nc.sync.dma_start(xt[:], xpack_d[i])
                xts.append(xt)

            # u = w1 @ xT, accumulated over the 32 input tiles as they land
            u_ps = [upsum.tile([128, TPC], dt.float32, tag=f"u{lh}",
                               name=f"ups{lh}") for lh in range(2)]
            for i in range(NXT):
                for lh in range(2):
                    s = i * 2 + lh
                    nc.tensor.matmul(u_ps[lh][:],
                                     w1t[:, s * 128:(s + 1) * 128],
                                     xts[i][:],
                                     start=(i == 0), stop=(i == NXT - 1))
            u_sb = []
            for lh in range(2):
                ut = res_pool.tile([128, TPC], dt.float16, tag=f"usb{lh}",
                                   name=f"usb{lh}")
                nc.vector.tensor_copy(ut[:], u_ps[lh][:])
                u_sb.append(ut)

            for g in range(NG):
                gt = gpool.tile([128, NSLOT * 128], dt.float16, tag="g",
                                name=f"gt{g}")
                nc.sync.dma_start(gt[:], g_d[g])
                acc = gpsum.tile([128, TPC], dt.float32, tag="acc",
                                 name=f"acc{g}")
                for s in range(10):
                    nc.tensor.matmul(acc[:], gt[:, s * 128:(s + 1) * 128],
                                     xts[int(xtile_idx[g, s])][:],
                                     start=(s == 0), stop=False)
                for lh in range(2):
                    nc.tensor.matmul(acc[:],
                                     gt[:, (10 + lh) * 128:(11 + lh) * 128],
                                     u_sb[lh][:],
                                     start=False, stop=(lh == 1))
                yt = ypool.tile([128, TPC], dt.float16, tag="y",
                                name=f"yt{g}")
                nc.vector.tensor_scalar_add(yt[:], acc[:], bt[:, g:g + 1])
                nc.sync.dma_start(y_d[g], yt[:])

    nc.compile()
    return nc


def _pack_weights(weight, w1, w2, b, flat):
    r2 = np.arange(BLOCK)
    gpack = np.empty((NG, 128, NSLOT * 128), np.float16)
    for ob in range(NB):
        for j in range(ACTIVE):
            m = int(flat[ob, j])
            q, a2 = m // ACTIVE, m % ACTIVE
            k = a2 * BLOCK + r2
            Wblk = weight[q * BLOCK + k // ACTIVE, k % ACTIVE, :]  # [r2, c]
            for rh in range(2):
                g = ob * 2 + rh
                for kh in range(2):
                    s = j * 2 + kh
                    gpack[g, :, s * 128:(s + 1) * 128] = \
                        Wblk[rh * 128:(rh + 1) * 128,
                             kh * 128:(kh + 1) * 128].T
    for g in range(NG):
        for lh in range(2):
            s = 10 + lh
            gpack[g, :, s * 128:(s + 1) * 128] = \
                w2[g * 128:(g + 1) * 128, lh * 128:(lh + 1) * 128].T
    w1sb = np.ascontiguousarray(
        w1.reshape(2, 128, 32, 128).transpose(2, 0, 3, 1)
          .reshape(64, 128, 128).transpose(1, 0, 2)
          .reshape(128, 64 * 128)).astype(np.float16)
    bpack = np.ascontiguousarray(b.reshape(NG, 128).T)
    return gpack, w1sb, bpack


def _ensure_axon_hooks():
    # Some images lack antenv.axon_hooks; bass_utils imports it on the
    # trace path. Provide a stub so trace degrades gracefully.
    import sys
    import types
    try:
        import antenv.axon_hooks  # noqa: F401
        return
    except ImportError:
        pass
    mod = types.ModuleType("antenv.axon_hooks")
    mod._hook = None
    mod.set_axon_ntff_profile_hook = lambda h: setattr(mod, "_hook", h)
    mod.get_axon_ntff_profile_hook = lambda: mod._hook
    sys.modules["antenv.axon_hooks"] = mod
    try:
        import antenv
        antenv.axon_hooks = mod
    except ImportError:
        pass


def kernel(x, weight, w1, w2, b, butterfly_flat_indices):
    _ensure_axon_hooks()
    from concourse.bass_utils import run_bass_kernel_spmd

    x = np.ascontiguousarray(x, np.float32)
    weight = np.ascontiguousarray(weight, np.float32)
    w1 = np.ascontiguousarray(w1, np.float32)
    w2 = np.ascontiguousarray(w2, np.float32)
    b = np.ascontiguousarray(b, np.float32)
    flat = np.asarray(butterfly_flat_indices)

    xtile_idx = _derive_xtile_idx(flat)
    key = xtile_idx.tobytes()
    if key not in _CACHE:
        _CACHE[key] = _build(xtile_idx)
    nc = _CACHE[key]

    gpack, w1sb, bpack = _pack_weights(weight, w1, w2, b, flat)
    in_maps = []
    for c in range(NCORES):
        xs = x[c * TPC:(c + 1) * TPC]
        xpack = np.ascontiguousarray(
            xs.T.reshape(NXT, 128, TPC)).astype(np.float16)
        in_maps.append({"xpack": xpack, "w1pack": w1sb, "gpack": gpack,
                        "bpack": bpack})

    trace = bool(int(os.environ.get("PIXELFLY_TRACE", "0")))
    res = run_bass_kernel_spmd(nc, in_maps, list(range(NCORES)), trace=trace)
    LAST["exec_time_ns"] = res.exec_time_ns
    LAST["results"] = res

    out = np.empty((TOKENS, OUT_F), np.float32)
    for c in range(NCORES):
        yc = res.results[c]["y"]  # [NG, 128, TPC] fp16
        out[c * TPC:(c + 1) * TPC] = \
            yc.reshape(OUT_F, TPC).T.astype(np.float32)
    return out


# revision 17
# speedup vs baseline: 1.1971x; 1.0169x over previous
import os
import numpy as np

# nn_PixelflyLinear: y = (x @ w1.T) @ w2.T + b + butterfly_matmul(x, weight, flat_idx)
# Data-parallel over tokens: 8 cores x 512 tokens, weights replicated.
# Device computes yT (out_f on partitions, tokens on free dim); host transposes.

TOKENS, IN_F, OUT_F, LOWRANK = 4096, 4096, 4096, 256
BLOCK, ACTIVE, NB = 256, 5, 16
NCORES = 8
TPC = TOKENS // NCORES          # 512 tokens per core
NG = OUT_F // 128               # 32 output half-block groups
NXT = IN_F // 128               # 32 input tiles
NSLOT = 12                      # 10 butterfly + 2 lowrank lhsT slots per group

_CACHE = {}
LAST = {"exec_time_ns": None}


def _derive_xtile_idx(flat):
    xtile_idx = np.zeros((NG, 10), np.int64)
    for ob in range(NB):
        for j in range(ACTIVE):
            m = int(flat[ob, j])
            q = m // ACTIVE
            for rh in range(2):
                for kh in range(2):
                    xtile_idx[ob * 2 + rh, j * 2 + kh] = q * 2 + kh
    return xtile_idx


def _build(xtile_idx):
    import concourse.bacc as bacc
    import concourse.mybir as mybir
    import concourse.tile as tile

    nc = bacc.Bacc("TRN2", target_bir_lowering=False, debug=False,
                   num_devices=NCORES)
    dt = mybir.dt

    LEADS = 6
    GP_AVAIL = [3, 5, 8, 10, 12, 14]

    xpack_d = nc.dram_tensor("xpack", [NXT, 128, TPC], dt.float16,
                             kind="ExternalInput")
    w1_d = nc.dram_tensor("w1pack", [4, 128, 16 * 128], dt.float16,
                          kind="ExternalInput")
    g_d = nc.dram_tensor("gpack", [NG, 128, NSLOT * 128], dt.float16,
                         kind="ExternalInput")
    b_d = nc.dram_tensor("bpack", [128, NG], dt.float32, kind="ExternalInput")
    y_d = nc.dram_tensor("y", [NG, 128, TPC], dt.float16,
                         kind="ExternalOutput")

    with tile.TileContext(nc) as tc:
        with (
            tc.tile_pool(name="res", bufs=1) as res_pool,
            tc.tile_pool(name="gstream", bufs=6) as gpool,
            tc.tile_pool(name="ypool", bufs=6) as ypool,
            tc.tile_pool(name="upsum", bufs=1, space="PSUM") as upsum,
            tc.tile_pool(name="gpsum", bufs=6, space="PSUM") as gpsum,
        ):
            bt = res_pool.tile([128, NG], dt.float32, tag="b")
            nc.sync.dma_start(bt[:], b_d[:])

            w1q = [None] * 4
            xts = [None] * NXT
            gts = [None] * NG
            accs = [None] * NG

            def dma_x(i):
                xt = res_pool.tile([128, TPC], dt.float16, tag=f"x{i}",
                                   name=f"xt{i}")
                nc.sync.dma_start(xt[:], xpack_d[i])
                xts[i] = xt

            def dma_w1(k):
                t = res_pool.tile([128, 16 * 128], dt.float16, tag=f"w1_{k}",
                                  name=f"w1q{k}")
                nc.sync.dma_start(t[:], w1_d[k])
                w1q[k] = t

            def dma_g(g):
                gt = gpool.tile([128, NSLOT * 128], dt.float16, tag="g",
                                name=f"gt{g}")
                nc.sync.dma_start(gt[:], g_d[g])
                gts[g] = gt

            # DMA issue order: w1 quarters + lead gpacks woven into x stream
            order = ["w1:0", "x:0", "g:0", "x:1", "x:2", "g:1", "x:3", "x:4",
                     "w1:1", "g:2", "x:5", "x:6", "g:3", "x:7", "x:8",
                     "w1:2", "g:4", "x:9", "x:10", "g:5", "x:11", "x:12",
                     "w1:3"] + [f"x:{i}" for i in range(13, NXT)]
            for item in order:
                kind, idx = item.split(":")
                {"x": dma_x, "w1": dma_w1, "g": dma_g}[kind](int(idx))

            u_ps = [upsum.tile([128, TPC], dt.float32, tag=f"u{lh}",
                               name=f"ups{lh}") for lh in range(2)]

            # merged emission: u matmuls + lead-group butterfly matmuls,
            # sorted by the x-tile index that unblocks them
            events = []
            for i in range(NXT):
                events.append((i, 0, ("u", i)))
            for g in range(LEADS):
                slots = sorted(
                    range(10),
                    key=lambda s: (max(int(xtile_idx[g, s]), GP_AVAIL[g]), s))
                first = True
                for s in slots:
                    av = max(int(xtile_idx[g, s]), GP_AVAIL[g])
                    events.append((av, 1, ("bf", g, s, first)))
                    first = False
            events.sort(key=lambda e: (e[0], e[1]))

            for av, pri, ev in events:
                if ev[0] == "u":
                    i = ev[1]
                    for lh in range(2):
                        sl = (i % 8) * 2 + lh
                        nc.tensor.matmul(u_ps[lh][:],
                                         w1q[i // 8][:, sl * 128:(sl + 1) * 128],
                                         xts[i][:],
                                         start=(i == 0), stop=(i == NXT - 1))
                else:
                    _, g, s, first = ev
                    if accs[g] is None:
                        accs[g] = gpsum.tile([128, TPC], dt.float32,
                                             tag="acc", name=f"acc{g}")
                    nc.tensor.matmul(accs[g][:],
                                     gts[g][:, s * 128:(s + 1) * 128],
                                     xts[int(xtile_idx[g, s])][:],
                                     start=first, stop=False)

            u_sb = []
            for lh in range(2):
                ut = res_pool.tile([128, TPC], dt.float16, tag=f"usb{lh}",
                                   name=f"usb{lh}")
                nc.vector.tensor_copy(ut[:], u_ps[lh][:])
                u_sb.append(ut)

            def close_group(g):
                for lh in range(2):
                    nc.tensor.matmul(accs[g][:],
                                     gts[g][:, (10 + lh) * 128:(11 + lh) * 128],
                                     u_sb[lh][:],
                                     start=False, stop=(lh == 1))
                yt = ypool.tile([128, TPC], dt.float16, tag="y",
                                name=f"yt{g}")
                nc.vector.tensor_scalar_add(yt[:], accs[g][:], bt[:, g:g + 1])
                nc.sync.dma_start(y_d[g], yt[:])

            for g in range(LEADS):
                close_group(g)

            for g in range(LEADS, NG):
                dma_g(g)
                accs[g] = gpsum.tile([128, TPC], dt.float32, tag="acc",
                                     name=f"acc{g}")
                for s in range(10):
                    nc.tensor.matmul(accs[g][:],
                                     gts[g][:, s * 128:(s + 1) * 128],
                                     xts[int(xtile_idx[g, s])][:],
                                     start=(s == 0), stop=False)
                close_group(g)

    nc.compile()
    return nc


def _pack_weights(weight, w1, w2, b, flat):
    r2 = np.arange(BLOCK)
    gpack = np.empty((NG, 128, NSLOT * 128), np.float16)
    for ob in range(NB):
        for j in range(ACTIVE):
            m = int(flat[ob, j])
            q, a2 = m // ACTIVE, m % ACTIVE
            k = a2 * BLOCK + r2
            Wblk = weight[q * BLOCK + k // ACTIVE, k % ACTIVE, :]  # [r2, c]
            for rh in range(2):
                g = ob * 2 + rh
                for kh in range(2):
                    s = j * 2 + kh
                    gpack[g, :, s * 128:(s + 1) * 128] = \
                        Wblk[rh * 128:(rh + 1) * 128,
                             kh * 128:(kh + 1) * 128].T
    for g in range(NG):
        for lh in range(2):
            s = 10 + lh
            gpack[g, :, s * 128:(s + 1) * 128] = \
                w2[g * 128:(g + 1) * 128, lh * 128:(lh + 1) * 128].T
    w1sb = (w1.reshape(2, 128, 32, 128).transpose(2, 0, 3, 1)
              .reshape(64, 128, 128).transpose(1, 0, 2)
              .reshape(128, 64 * 128))
    w1sb = np.ascontiguousarray(
        w1sb.reshape(128, 4, 16 * 128).transpose(1, 0, 2)).astype(np.float16)
    bpack = np.ascontiguousarray(b.reshape(NG, 128).T)
    return gpack, w1sb, bpack


def _ensure_axon_hooks():
    # Some images lack antenv.axon_hooks; bass_utils imports it on the
    # trace path. Provide a stub so trace degrades gracefully.
    import sys
    import types
    try:
        import antenv.axon_hooks  # noqa: F401
        return
    except ImportError:
        pass
    mod = types.ModuleType("antenv.axon_hooks")
    mod._hook = None
    mod.set_axon_ntff_profile_hook = lambda h: setattr(mod, "_hook", h)
    mod.get_axon_ntff_profile_hook = lambda: mod._hook
    sys.modules["antenv.axon_hooks"] = mod
    try:
        import antenv
        antenv.axon_hooks = mod
    except ImportError:
        pass


def kernel(x, weight, w1, w2, b, butterfly_flat_indices):
    _ensure_axon_hooks()
    from concourse.bass_utils import run_bass_kernel_spmd

    x = np.ascontiguousarray(x, np.float32)
    weight = np.ascontiguousarray(weight, np.float32)
    w1 = np.ascontiguousarray(w1, np.float32)
    w2 = np.ascontiguousarray(w2, np.float32)
    b = np.ascontiguousarray(b, np.float32)
    flat = np.asarray(butterfly_flat_indices)

    xtile_idx = _derive_xtile_idx(flat)
    key = xtile_idx.tobytes()
    if key not in _CACHE:
        _CACHE[key] = _build(xtile_idx)
    nc = _CACHE[key]

    gpack, w1sb, bpack = _pack_weights(weight, w1, w2, b, flat)
    in_maps = []
    for c in range(NCORES):
        xs = x[c * TPC:(c + 1) * TPC]
        xpack = np.ascontiguousarray(
            xs.T.reshape(NXT, 128, TPC)).astype(np.float16)
        in_maps.append({"xpack": xpack, "w1pack": w1sb, "gpack": gpack,
                        "bpack": bpack})

    trace = bool(int(os.environ.get("PIXELFLY_TRACE", "0")))
    res = run_bass_kernel_spmd(nc, in_maps, list(range(NCORES)), trace=trace)
    LAST["exec_time_ns"] = res.exec_time_ns
    LAST["results"] = res

    out = np.empty((TOKENS, OUT_F), np.float32)
    for c in range(NCORES):
        yc = res.results[c]["y"]  # [NG, 128, TPC] fp16
        out[c * TPC:(c + 1) * TPC] = \
            yc.reshape(OUT_F, TPC).T.astype(np.float32)
    return out
